# revision 2
# baseline (speedup 1.0000x reference)
"""v12-final (v10 lineage): fp16 numerics (v7) + x SBUF-resident + software-pipelined engine
balance.  The per-lk attention chain (score -> exp -> AV) is ACT-paced
(exp 1049ns vs 854ns of PE work), so the PE would stall ~200ns per lk.
Fix: interleave the NEXT pass's projection matmuls (or the previous
chunk's o-projection) between attention iterations as filler PE work,
driven by generators yielding one matmul at a time.

Rope: one psum->fp16 copy, then all-SBUF fp16 muls/adds (DVE 2x/4x
modes).  o-proj psum->sbuf copies on gpsimd; exp entirely on ACT.
"""

import os
import sys

for _p in ("/opt/trn_rl_repo", "/root/.axon_site/_ro/trn_rl_repo"):
    if os.path.isdir(_p) and _p not in sys.path:
        sys.path.insert(0, _p)

import contextlib

import numpy as np

import concourse.bass as bass
import concourse.tile as tile
from concourse import bacc, mybir
from concourse.bass_utils import run_bass_kernel_spmd

P = 128
L = 2048
D = 1536
HL = 6
HD = 64
EQ = 384
NQK = 768
DC = D // P      # 12
LT = L // P      # 16
ACH = 512        # attention lq chunk
XCH = 512        # qkv l chunk
F32 = mybir.dt.float32
F16 = mybir.dt.float16
AF = mybir.ActivationFunctionType


def build_bass(repeat=1):
    nc = bacc.Bacc("TRN2", target_bir_lowering=False, debug=False, num_devices=8)
    xT = nc.dram_tensor("xT", [D, L], F16, kind="ExternalInput")
    wqkT = nc.dram_tensor("wqkT", [D, NQK], F16, kind="ExternalInput")
    wvT = nc.dram_tensor("wvT", [D, EQ], F16, kind="ExternalInput")
    woT = nc.dram_tensor("woT", [EQ, D], F16, kind="ExternalInput")
    cos2 = nc.dram_tensor("cos2", [P, L], F16, kind="ExternalInput")
    ss2 = nc.dram_tensor("ss2", [P, L], F16, kind="ExternalInput")
    out = nc.dram_tensor("out", [L, D], F32, kind="ExternalOutput")

    xT_r = xT.rearrange("(dc p) l -> p dc l", p=P)
    wqkT_r = wqkT.rearrange("(dc p) e -> p dc e", p=P)
    wvT_r = wvT.rearrange("(dc p) e -> p dc e", p=P)
    woT_r = woT.rearrange("(ec p) d -> p ec d", p=P)

    with tile.TileContext(nc) as tc:
        rep_cm = tc.For_i(0, repeat, 1) if repeat > 1 else contextlib.nullcontext()
        with rep_cm, tc.tile_pool(name="persist", bufs=1) as persist:
            qT = persist.tile([P, 3, L], F16)
            kT = persist.tile([P, 3, L], F16)
            v1 = persist.tile([P, LT, HL, HD + 1], F16)
            xfull = persist.tile([P, DC, L], F16)
            cos_sb = persist.tile([P, L], F16)
            ss_sb = persist.tile([P, L], F16)
            outT = persist.tile([P, 3, L], F16)

            with (
                tc.tile_pool(name="s2w", bufs=2) as s2w,
                tc.tile_pool(name="s2t", bufs=2) as s2t,
                tc.tile_pool(name="s2att", bufs=2) as s2att,
                tc.tile_pool(name="s2o", bufs=3) as s2o,
                tc.tile_pool(name="s2nrm", bufs=3) as s2nrm,
                tc.tile_pool(name="ps_acc", bufs=2, space=bass.MemorySpace.PSUM) as ps_acc,
                tc.tile_pool(name="ps_s", bufs=2, space=bass.MemorySpace.PSUM) as ps_s,
                tc.tile_pool(name="ps_av", bufs=2, space=bass.MemorySpace.PSUM) as ps_av,
            ):
                wtiles = {}

                def load_wqk(etp):
                    wq = s2w.tile([P, DC, EQ], F16, tag="w")
                    wtiles[etp] = wq
                    for d0 in range(0, DC, 3):
                        dsl = slice(d0, d0 + 3)
                        nc.sync.dma_start(
                            wq[:, dsl, 0:P], wqkT_r[:, dsl, etp * P : (etp + 1) * P]
                        )
                        nc.sync.dma_start(
                            wq[:, dsl, P : 2 * P],
                            wqkT_r[:, dsl, EQ + etp * P : EQ + (etp + 1) * P],
                        )

                def proj_chunk_gen(etp, c, with_v):
                    """qk projection + rope for token chunk c; yields after
                    each PE matmul so attention can interleave."""
                    sl = slice(c * XCH, (c + 1) * XCH)
                    wq = wtiles[etp]
                    for half in range(2):
                        ps = ps_acc.tile([P, ACH], F32, tag="acc")
                        for dc in range(DC):
                            nc.tensor.matmul(
                                ps[:, 0:XCH],
                                wq[:, dc, half * P : (half + 1) * P],
                                xfull[:, dc, sl],
                                start=(dc == 0),
                                stop=(dc == DC - 1),
                            )
                            yield
                        t16 = s2t.tile([P, XCH], F16, tag="t16")
                        nc.vector.tensor_copy(t16[:], ps[:, 0:XCH])
                        tcos = s2t.tile([P, XCH], F16, tag="tcos")
                        nc.vector.tensor_mul(tcos[:], t16[:], cos_sb[:, sl])
                        trot = s2t.tile([P, XCH], F16, tag="trot")
                        for q_ in range(4):
                            s = (q_ ^ 1) * 32
                            d_ = q_ * 32
                            nc.vector.tensor_mul(
                                trot[d_ : d_ + 32, :],
                                t16[s : s + 32, :],
                                ss_sb[s : s + 32, sl],
                            )
                        dst = (qT if half == 0 else kT)[:, etp, sl]
                        nc.vector.tensor_add(dst, tcos[:], trot[:])
                        yield
                    if with_v:
                        for lt2 in range(XCH // P):
                            lk = c * (XCH // P) + lt2
                            pv = ps_acc.tile([P, ACH], F32, tag="acc")
                            for dc in range(DC):
                                nc.tensor.matmul(
                                    pv[:, 0:EQ],
                                    xfull[:, dc, c * XCH + lt2 * P : c * XCH + (lt2 + 1) * P],
                                    wv_sb[:, dc, :],
                                    start=(dc == 0),
                                    stop=(dc == DC - 1),
                                )
                                yield
                            nc.scalar.copy(
                                v1[:, lk, :, 0:HD],
                                pv[:, 0:EQ].rearrange("p (h d) -> p h d", h=HL),
                            )
                            yield

                def oproj_chunk_gen(cq):
                    """o-projection for lq chunk cq; yields after each PE op."""
                    for lt in range(ACH // P):
                        l0 = cq * ACH + lt * P
                        for dn in range(D // ACH):
                            pso = ps_acc.tile([P, ACH], F32, tag="acc")
                            for ec in range(3):
                                nc.tensor.matmul(
                                    pso[:],
                                    outT[:, ec, l0 : l0 + P],
                                    wo_sb[:, ec, dn * ACH : (dn + 1) * ACH],
                                    start=(ec == 0),
                                    stop=(ec == 2),
                                )
                                yield
                            ot = s2o.tile([P, ACH], F32)
                            nc.vector.tensor_copy(ot[:], pso[:])
                            hw = ACH // 2
                            for hx in range(2):
                                nc.sync.dma_start(
                                    out[
                                        l0 : l0 + P,
                                        dn * ACH + hx * hw : dn * ACH + (hx + 1) * hw,
                                    ],
                                    ot[:, hx * hw : (hx + 1) * hw],
                                )
                            yield

                def drain(gen, n):
                    if gen is None:
                        return
                    for _ in range(n):
                        if next(gen, "END") == "END":
                            break

                def attention_cq(etp, cq, filler, fill_rate):
                    cqs = slice(cq * ACH, (cq + 1) * ACH)
                    pav0 = ps_av.tile([HD + 1, ACH], F32, tag="av")
                    pav1 = ps_av.tile([HD + 1, ACH], F32, tag="av")
                    for lk in range(LT):
                        pscore = ps_s.tile([P, 2 * ACH], F32)
                        att = s2att.tile([P, 2 * ACH], F16)
                        for hh in range(2):
                            po = hh * HD
                            nc.tensor.matmul(
                                pscore[:, hh * ACH : (hh + 1) * ACH],
                                kT[po : po + HD, etp, lk * P : (lk + 1) * P],
                                qT[po : po + HD, etp, cqs],
                                start=True,
                                stop=True,
                            )
                        nc.scalar.activation(att[:], pscore[:], AF.Exp, scale=0.125)
                        drain(filler, fill_rate)
                        for hh, pav in ((0, pav0), (1, pav1)):
                            nc.tensor.matmul(
                                pav[:],
                                v1[:, lk, 2 * etp + hh, :],
                                att[:, hh * ACH : (hh + 1) * ACH],
                                start=(lk == 0),
                                stop=(lk == LT - 1),
                            )
                    for hh, pav in ((0, pav0), (1, pav1)):
                        po = hh * HD
                        u = s2nrm.tile([HD, ACH], F32, tag="u")
                        nc.vector.tensor_copy(u[:], pav[0:HD, :])
                        dcp = s2nrm.tile([1, ACH], F32, tag="dcp")
                        nc.vector.tensor_copy(dcp[:], pav[HD : HD + 1, :])
                        rcp = s2nrm.tile([1, ACH], F32, tag="rcp")
                        nc.vector.reciprocal_approx_fast(out=rcp[:], in_=dcp[:])
                        rb = s2nrm.tile([HD, ACH], F32, tag="rb")
                        nc.gpsimd.partition_broadcast(rb[:], rcp[:], channels=HD)
                        nc.vector.tensor_mul(
                            outT[po : po + HD, etp, cqs], u[:], rb[:]
                        )

                # ---- pipeline ----
                # startup order: the first proj matmul only needs wqk0's dc
                # group 0 and x chunk0 -- issue those DMAs before anything
                # else so the PE starts ~5us in, not ~24us.
                wq0 = s2w.tile([P, DC, EQ], F16, tag="w")
                wtiles[0] = wq0
                for d0 in range(0, DC, 3):
                    dsl = slice(d0, d0 + 3)
                    nc.sync.dma_start(
                        wq0[:, dsl, 0:P], wqkT_r[:, dsl, 0:P]
                    )
                    nc.sync.dma_start(
                        wq0[:, dsl, P : 2 * P], wqkT_r[:, dsl, EQ : EQ + P]
                    )
                    nc.sync.dma_start(
                        xfull[:, dsl, 0:XCH], xT_r[:, dsl, 0:XCH]
                    )
                ones_c = nc.const_aps.tensor(1.0, (P, 1), F32)
                nc.vector.tensor_copy(
                    v1[:, :, :, HD : HD + 1], ones_c.to_broadcast([P, LT, HL, 1])
                )
                nc.sync.dma_start(cos_sb[:], cos2[:])
                nc.sync.dma_start(ss_sb[:], ss2[:])
                wv_sb = s2w.tile([P, DC, EQ], F16, tag="w")
                for d0 in range(0, DC, 3):
                    nc.sync.dma_start(
                        wv_sb[:, d0 : d0 + 3, :], wvT_r[:, d0 : d0 + 3, :]
                    )

                # etp0 projection + V, standalone (nothing to overlap yet)
                for c in range(L // XCH):
                    if c > 0:
                        for d0 in range(0, DC, 3):
                            nc.sync.dma_start(
                                xfull[:, d0 : d0 + 3, c * XCH : (c + 1) * XCH],
                                xT_r[:, d0 : d0 + 3, c * XCH : (c + 1) * XCH],
                            )
                    drain(proj_chunk_gen(0, c, with_v=True), 10**9)

                load_wqk(1)
                for cq in range(L // ACH):
                    attention_cq(0, cq, proj_chunk_gen(1, cq, with_v=False), 2)

                load_wqk(2)
                for cq in range(L // ACH):
                    attention_cq(1, cq, proj_chunk_gen(2, cq, with_v=False), 2)

                wo_sb = s2w.tile([P, 3, D], F16, tag="w")
                nc.sync.dma_start(wo_sb[:], woT_r[:])
                prev_o = None
                for cq in range(L // ACH):
                    attention_cq(2, cq, prev_o, 3)
                    drain(prev_o, 10**9)
                    prev_o = oproj_chunk_gen(cq)
                drain(prev_o, 10**9)

    nc.compile()
    return nc


_NC_CACHE = None


def _get_nc():
    global _NC_CACHE
    if _NC_CACHE is None:
        _NC_CACHE = build_bass()
    return _NC_CACHE


def make_in_maps(x, w_qkv, w_o, cos, sin):
    x = np.asarray(x, dtype=np.float32)
    w_qkv = np.asarray(w_qkv, dtype=np.float32)
    w_o = np.asarray(w_o, dtype=np.float32)
    cos = np.asarray(cos, dtype=np.float32)
    sin = np.asarray(sin, dtype=np.float32)

    cosT = np.ascontiguousarray(cos.T)
    sinT = sin.T
    # pre-swapped so the trot muls read t16 and ss at the SAME base
    # partition (SBUF+SBUF DVE inputs must share base); row r holds the
    # sin factor needed by output row r^32.
    ssp = np.concatenate([sinT[32:64], -sinT[0:32]], axis=0)
    cos2 = np.ascontiguousarray(np.tile(cosT, (2, 1))).astype(np.float16)
    ss2 = np.ascontiguousarray(np.tile(ssp, (2, 1))).astype(np.float16)

    in_maps = []
    for c in range(8):
        b, g = c // 4, c % 4
        xTc = np.ascontiguousarray(x[b].T).astype(np.float16)
        wq = w_qkv[g * EQ : (g + 1) * EQ]
        wk = w_qkv[D + g * EQ : D + (g + 1) * EQ]
        wv = w_qkv[2 * D + g * EQ : 2 * D + (g + 1) * EQ]
        wqkTc = np.ascontiguousarray(np.concatenate([wq, wk], 0).T).astype(np.float16)
        wvTc = np.ascontiguousarray(wv.T).astype(np.float16)
        woTc = np.ascontiguousarray(w_o[:, g * EQ : (g + 1) * EQ].T).astype(np.float16)
        in_maps.append(
            {
                "xT": xTc,
                "wqkT": wqkTc,
                "wvT": wvTc,
                "woT": woTc,
                "cos2": cos2,
                "ss2": ss2,
            }
        )
    return in_maps


def kernel(x, w_qkv, w_o, cos, sin):
    nc = _get_nc()
    in_maps = make_in_maps(x, w_qkv, w_o, cos, sin)
    res = run_bass_kernel_spmd(nc, in_maps, core_ids=list(range(8)))
    outs = [res.results[c]["out"] for c in range(8)]
    full = np.stack(
        [
            outs[0] + outs[1] + outs[2] + outs[3],
            outs[4] + outs[5] + outs[6] + outs[7],
        ]
    ).astype(np.float32)
    return full
